# revision 4
# baseline (speedup 1.0000x reference)
"""Grouped-experts MoE MLP (Aria) on 8 TRN2 NeuronCores.

Expert parallelism: 8 experts / 8 cores -> each core owns one expert's
weights (w1 [2048, 8192], w2 [4096, 2048]) and processes that expert's
token block (tokens are pre-sorted by expert, so routing is host-side
slicing). No device collectives needed.

Per-core device kernel (all matmul compute in bf16, fp32 accumulate),
structured to keep the PE instruction stream small (~840 instructions;
this environment executes large unrolled streams at ~1.8us/instruction
past ~1.5k instructions — instruction fetch is the cliff):

  phase 1 (tokens on PSUM partitions, fat N=512 matmuls):
      fc1[tok, icol-tile] = sum_k xt[k][:, tok].T @ w1[k, icol-tile]
      proj/gate tiles paired; hidden[tok, :] = silu(proj) * gate (bf16)
  transpose: 32 PE transposes hidden -> hT [inter-p, tok]
  phase 2: out[tok, hcol] = sum_j hT[j].T @ w2[j, hcol]
      4 PSUM accumulators [128, 512] over 32 inter k-tiles

Host pre-arranges weight layouts so every DMA is partition-major with
>=16KB contiguous per partition, and casts to bf16 (halves the HBM
traffic; the memory-bound roofline is weight streaming).
"""

import sys
import types

sys.path.insert(0, "/opt/trn_rl_repo")

# This axon deployment ships without antenv.axon_hooks; shim it so
# bass_utils' trace path degrades gracefully instead of ImportError-ing.
try:
    import antenv  # noqa: F401

    if "antenv.axon_hooks" not in sys.modules:
        _hooks = types.ModuleType("antenv.axon_hooks")
        _hooks.get_axon_ntff_profile_hook = lambda: None
        sys.modules["antenv.axon_hooks"] = _hooks
except ImportError:
    pass

from contextlib import ExitStack

import ml_dtypes
import numpy as np

import concourse.bass as bass  # noqa: F401
import concourse.tile as tile
from concourse import bacc, mybir
from concourse.bass_utils import run_bass_kernel_spmd
from concourse.masks import make_identity

NUM_TOKENS = 1024
HIDDEN = 2048
INTER = 4096
EXPERTS = 8
N_CORES = 8
P = 128
T = 128  # tokens per core (padded)
KT1 = HIDDEN // P  # 16 k-tiles for matmul 1
NT1 = (2 * INTER) // 512  # 16 fc1 column tiles of 512
JT = INTER // P  # 32 inter k-tiles for matmul 2
NT2 = HIDDEN // 512  # 4 output column tiles of 512

BF16 = mybir.dt.bfloat16
F32 = mybir.dt.float32

_CACHE = {}


def _build(reps: int = 1):
    nc = bacc.Bacc(
        "TRN2", target_bir_lowering=False, debug=False, num_devices=N_CORES
    )
    xt_d = nc.dram_tensor("xt", [P, KT1 * T], BF16, kind="ExternalInput").ap()
    w1_d = nc.dram_tensor(
        "w1", [P, NT1, KT1 * 512], BF16, kind="ExternalInput"
    ).ap()
    w2_d = nc.dram_tensor("w2", [P, JT, HIDDEN], BF16, kind="ExternalInput").ap()
    out_d = nc.dram_tensor("out", [T, HIDDEN], F32, kind="ExternalOutput").ap()

    with tile.TileContext(nc) as tc:
        with ExitStack() as ctx:
            xpool = ctx.enter_context(tc.tile_pool(name="x", bufs=1))
            ipool = ctx.enter_context(tc.tile_pool(name="id", bufs=1))
            w1pool = ctx.enter_context(tc.tile_pool(name="w1", bufs=3))
            w2pool = ctx.enter_context(tc.tile_pool(name="w2", bufs=2))
            hpool = ctx.enter_context(tc.tile_pool(name="h", bufs=1))
            spool = ctx.enter_context(tc.tile_pool(name="s", bufs=2))
            opool = ctx.enter_context(tc.tile_pool(name="o", bufs=1))
            psum1 = ctx.enter_context(tc.tile_pool(name="ps1", bufs=3, space="PSUM"))
            trp = ctx.enter_context(tc.tile_pool(name="tr", bufs=1, space="PSUM"))
            psum2 = ctx.enter_context(tc.tile_pool(name="ps2", bufs=1, space="PSUM"))

            xt = xpool.tile([P, KT1 * T], BF16)
            nc.sync.dma_start(xt[:], xt_d[:, :])
            ident = ipool.tile([P, P], BF16)
            make_identity(nc, ident[:])

            for _rep in range(reps):
                hidden = hpool.tile([T, JT * P], BF16, tag="hid")
                hT = hpool.tile([P, JT * T], BF16, tag="hT")
                po = [psum2.tile([P, 512], F32, name=f"po{n}") for n in range(NT2)]

                # phase 1: np pairs (proj tile np, gate tile np+8)
                for np_ in range(NT1 // 2):
                    wp = w1pool.tile([P, KT1 * 512], BF16, tag="w1t")
                    nc.sync.dma_start(wp[:], w1_d[:, np_])
                    wg = w1pool.tile([P, KT1 * 512], BF16, tag="w1t")
                    nc.sync.dma_start(wg[:], w1_d[:, np_ + 8])

                    pa = psum1.tile([T, 512], F32, tag="ps1t")
                    pb = psum1.tile([T, 512], F32, tag="ps1t")
                    for k in range(KT1):
                        nc.tensor.matmul(
                            pa[:],
                            lhsT=xt[:, k * T : (k + 1) * T],
                            rhs=wp[:, k * 512 : (k + 1) * 512],
                            start=(k == 0),
                            stop=(k == KT1 - 1),
                        )
                    for k in range(KT1):
                        nc.tensor.matmul(
                            pb[:],
                            lhsT=xt[:, k * T : (k + 1) * T],
                            rhs=wg[:, k * 512 : (k + 1) * 512],
                            start=(k == 0),
                            stop=(k == KT1 - 1),
                        )
                    sa = spool.tile([T, 512], F32, tag="silu")
                    nc.scalar.activation(
                        sa[:], pa[:], mybir.ActivationFunctionType.Silu
                    )
                    nc.vector.tensor_mul(
                        hidden[:, np_ * 512 : (np_ + 1) * 512], sa[:], pb[:]
                    )

                # transpose hidden [tok, inter] -> hT [inter-p, tok]
                for i in range(JT):
                    tp = trp.tile([P, P], BF16, tag="trt")
                    nc.tensor.transpose(
                        tp[:], hidden[:, i * P : (i + 1) * P], ident[:]
                    )
                    nc.vector.tensor_copy(hT[:, i * T : (i + 1) * T], tp[:])

                # phase 2
                for jb in range(JT // 4):
                    w2t = w2pool.tile([P, 4 * HIDDEN], BF16, tag="w2t")
                    nc.sync.dma_start(w2t[:], w2_d[:, 4 * jb : 4 * jb + 4, :])
                    for u in range(4):
                        j = 4 * jb + u
                        for n in range(NT2):
                            nc.tensor.matmul(
                                po[n][:],
                                lhsT=hT[:, j * T : (j + 1) * T],
                                rhs=w2t[:, u * HIDDEN + n * 512 : u * HIDDEN + (n + 1) * 512],
                                start=(j == 0),
                                stop=(j == JT - 1),
                            )

                osb = opool.tile([T, HIDDEN], F32, tag="osb")
                for n in range(NT2):
                    nc.scalar.copy(osb[:, n * 512 : (n + 1) * 512], po[n][:])
                nc.sync.dma_start(out_d[:, :], osb[:])

    nc.compile()
    return nc


def _get_nc(reps: int = 1):
    key = ("nc", reps)
    if key not in _CACHE:
        _CACHE[key] = _build(reps)
    return _CACHE[key]


def _prep_token_block(x_block: np.ndarray) -> np.ndarray:
    """[T, HIDDEN] f32 -> xt layout [P, KT1*T] bf16 where
    xt[p, k*T + t] = x_block[t, k*P + p]."""
    a = np.ascontiguousarray(
        x_block.T.reshape(KT1, P, T).transpose(1, 0, 2).reshape(P, KT1 * T)
    )
    return a.astype(ml_dtypes.bfloat16)


def _prep_w1(w1_e: np.ndarray) -> np.ndarray:
    """[HIDDEN, 2*INTER] f32 -> [P, NT1, KT1*512] bf16 where
    [p, n, k*512 + c] = w1_e[k*P + p, n*512 + c]."""
    a = w1_e.reshape(KT1, P, NT1, 512).transpose(1, 2, 0, 3)  # [p, n, k, c]
    return np.ascontiguousarray(a.reshape(P, NT1, KT1 * 512)).astype(
        ml_dtypes.bfloat16
    )


def _prep_w2(w2_e: np.ndarray) -> np.ndarray:
    """[INTER, HIDDEN] f32 -> [P, JT, HIDDEN] bf16 where
    [p, j, c] = w2_e[j*P + p, c]."""
    return np.ascontiguousarray(
        w2_e.reshape(JT, P, HIDDEN).transpose(1, 0, 2)
    ).astype(ml_dtypes.bfloat16)


def _run_device(in_maps):
    nc = _get_nc()
    res = run_bass_kernel_spmd(nc, in_maps, core_ids=list(range(N_CORES)))
    return [r["out"] for r in res.results]


def kernel(permuted_tokens, w1, w2, tokens_per_expert):
    permuted_tokens = np.asarray(permuted_tokens, dtype=np.float32)
    w1 = np.asarray(w1, dtype=np.float32)
    w2 = np.asarray(w2, dtype=np.float32)
    counts = np.asarray(tokens_per_expert).astype(np.int64)

    n = permuted_tokens.shape[0]
    bounds = np.minimum(np.cumsum(counts), n)
    starts = np.concatenate([[0], bounds[:-1]])
    eff_counts = np.maximum(bounds - starts, 0)

    w1_maps = [_prep_w1(w1[e]) for e in range(EXPERTS)]
    w2_maps = [_prep_w2(w2[e]) for e in range(EXPERTS)]

    out = np.zeros((n, HIDDEN), dtype=np.float32)
    rounds = int(max(1, -(-int(eff_counts.max()) // T)))
    for r in range(rounds):
        in_maps = []
        chunk_info = []
        for e in range(EXPERTS):
            c0 = starts[e] + r * T
            cnt = int(min(max(eff_counts[e] - r * T, 0), T))
            blk = np.zeros((T, HIDDEN), dtype=np.float32)
            if cnt > 0:
                blk[:cnt] = permuted_tokens[c0 : c0 + cnt]
            chunk_info.append((c0, cnt))
            in_maps.append(
                {"xt": _prep_token_block(blk), "w1": w1_maps[e], "w2": w2_maps[e]}
            )
        outs = _run_device(in_maps)
        for e in range(EXPERTS):
            c0, cnt = chunk_info[e]
            if cnt > 0:
                out[c0 : c0 + cnt] = outs[e][:cnt]
    return out


# revision 5
# speedup vs baseline: 5.7172x; 5.7172x over previous
"""Grouped-experts MoE MLP (Aria) on 8 TRN2 NeuronCores.

Expert parallelism: 8 experts / 8 cores -> each core owns one expert's
weights (w1 [2048, 8192], w2 [4096, 2048]) and processes that expert's
token block (tokens are pre-sorted by expert, so routing is host-side
slicing). No device collectives needed.

This environment (axon-virtualized NeuronCores) executes long unrolled
instruction streams at ~1.8us/instruction past a few hundred matmuls,
so the kernel is one fused For_i hardware loop (8 iterations, ~104 PE
instructions per body — IRAM-resident):

  per iteration (one 512-wide fc1 column group np, bf16/f32-accum):
    DMA w1[:, proj np], w1[:, gate np], w2[4 j-tiles]      (6 MB)
    fc1 proj/gate psum [tok, 512] = sum_k xt[k].T @ w1[k]  (32 matmuls)
    hidden = silu(proj) * gate          (ACT + DVE, bf16)
    hT segs = PE-transpose(hidden)      (4 transposes via identity)
    out psum[n] += hT[u].T @ w2[j=4np+u, n]                (16 matmuls)

The out accumulators live in PSUM across the whole loop; they are
seeded by start=True matmuls on zeros before the loop (start flags
must be static inside a dynamic loop).

Host pre-arranges weights partition-major so every DMA is contiguous
per partition, and casts to bf16 (halves the HBM traffic; the
memory-bound roofline is weight streaming).
"""

import sys
import types

sys.path.insert(0, "/opt/trn_rl_repo")

# This axon deployment ships without antenv.axon_hooks; shim it so
# bass_utils' trace path degrades gracefully instead of ImportError-ing.
try:
    import antenv  # noqa: F401

    if "antenv.axon_hooks" not in sys.modules:
        _hooks = types.ModuleType("antenv.axon_hooks")
        _hooks.get_axon_ntff_profile_hook = lambda: None
        sys.modules["antenv.axon_hooks"] = _hooks
except ImportError:
    pass

from contextlib import ExitStack

import ml_dtypes
import numpy as np

import concourse.bass as bass  # noqa: F401
import concourse.tile as tile
from concourse import bacc, mybir
from concourse.bass import ds
from concourse.bass_utils import run_bass_kernel_spmd
from concourse.masks import make_identity

NUM_TOKENS = 1024
HIDDEN = 2048
INTER = 4096
EXPERTS = 8
N_CORES = 8
P = 128
T = 128  # tokens per core (padded)
KT1 = HIDDEN // P  # 16 k-tiles for matmul 1
NT1 = (2 * INTER) // 512  # 16 fc1 column tiles of 512
NP1 = NT1 // 2  # 8 proj/gate pair groups = loop trip count
JT = INTER // P  # 32 inter k-tiles for matmul 2
NT2 = HIDDEN // 512  # 4 output column tiles of 512
GCOL = KT1 * 512  # 8192 = columns per 512-icol group in w1 layout

BF16 = mybir.dt.bfloat16
F32 = mybir.dt.float32

_CACHE = {}


def _build(reps: int = 1):
    nc = bacc.Bacc(
        "TRN2", target_bir_lowering=False, debug=False, num_devices=N_CORES
    )
    xt_d = nc.dram_tensor("xt", [P, KT1 * T], BF16, kind="ExternalInput").ap()
    w1_d = nc.dram_tensor("w1", [P, NT1 * GCOL], BF16, kind="ExternalInput").ap()
    w2_d = nc.dram_tensor("w2", [P, JT * HIDDEN], BF16, kind="ExternalInput").ap()
    out_d = nc.dram_tensor("out", [T, HIDDEN], F32, kind="ExternalOutput").ap()

    with tile.TileContext(nc) as tc:
        with ExitStack() as ctx:
            xpool = ctx.enter_context(tc.tile_pool(name="x", bufs=1))
            ipool = ctx.enter_context(tc.tile_pool(name="id", bufs=1))
            w1pool = ctx.enter_context(tc.tile_pool(name="w1", bufs=2))
            w2pool = ctx.enter_context(tc.tile_pool(name="w2", bufs=1))
            spool = ctx.enter_context(tc.tile_pool(name="s", bufs=1))
            hpool = ctx.enter_context(tc.tile_pool(name="h", bufs=1))
            opool = ctx.enter_context(tc.tile_pool(name="o", bufs=1))
            psum1 = ctx.enter_context(tc.tile_pool(name="ps1", bufs=2, space="PSUM"))
            trp = ctx.enter_context(tc.tile_pool(name="tr", bufs=2, space="PSUM"))
            psum2 = ctx.enter_context(tc.tile_pool(name="ps2", bufs=1, space="PSUM"))

            xt = xpool.tile([P, KT1 * T], BF16)
            nc.sync.dma_start(xt[:], xt_d[:, :])
            ident = ipool.tile([P, P], BF16)
            make_identity(nc, ident[:])
            zt = ipool.tile([P, 512], BF16)
            nc.vector.memset(zt[:], 0.0)

            for _rep in range(reps):
                po = [psum2.tile([P, 512], F32, name=f"po{n}") for n in range(NT2)]
                # seed the accumulators (start flags are static in the loop)
                for n in range(NT2):
                    nc.tensor.matmul(
                        po[n][:],
                        lhsT=zt[:, :P],
                        rhs=zt[:, :512],
                        start=True,
                        stop=False,
                        skip_group_check=True,
                    )

                with tc.For_i(0, NP1, 1, staggered_reset=True) as np_:
                    wp = w1pool.tile([P, GCOL], BF16, tag="w1t")
                    nc.sync.dma_start(wp[:], w1_d[:, ds(np_ * GCOL, GCOL)])
                    wg = w1pool.tile([P, GCOL], BF16, tag="w1t")
                    nc.sync.dma_start(
                        wg[:], w1_d[:, ds(np_ * GCOL + NP1 * GCOL, GCOL)]
                    )
                    w2t = w2pool.tile([P, 4 * HIDDEN], BF16, tag="w2t")
                    nc.sync.dma_start(
                        w2t[:], w2_d[:, ds(np_ * (4 * HIDDEN), 4 * HIDDEN)]
                    )

                    pa = psum1.tile([T, 512], F32, tag="ps1t")
                    pb = psum1.tile([T, 512], F32, tag="ps1t")
                    for k in range(KT1):
                        nc.tensor.matmul(
                            pa[:],
                            lhsT=xt[:, k * T : (k + 1) * T],
                            rhs=wp[:, k * 512 : (k + 1) * 512],
                            start=(k == 0),
                            stop=(k == KT1 - 1),
                        )
                    for k in range(KT1):
                        nc.tensor.matmul(
                            pb[:],
                            lhsT=xt[:, k * T : (k + 1) * T],
                            rhs=wg[:, k * 512 : (k + 1) * 512],
                            start=(k == 0),
                            stop=(k == KT1 - 1),
                        )
                    sa = spool.tile([T, 512], F32, tag="silu")
                    nc.scalar.activation(
                        sa[:], pa[:], mybir.ActivationFunctionType.Silu
                    )
                    hseg = hpool.tile([T, 512], BF16, tag="hseg")
                    nc.vector.tensor_mul(hseg[:], sa[:], pb[:])

                    # transpose the 4 [128,128] sub-tiles of hseg
                    hsegT = hpool.tile([P, 4 * T], BF16, tag="hsegT")
                    for half in range(2):
                        tp = trp.tile([P, 2 * P], BF16, tag="trt")
                        for s in range(2):
                            u = 2 * half + s
                            nc.tensor.transpose(
                                tp[:, s * P : (s + 1) * P],
                                hseg[:, u * P : (u + 1) * P],
                                ident[:],
                            )
                        nc.vector.tensor_copy(
                            hsegT[:, half * 2 * T : (half + 1) * 2 * T], tp[:]
                        )

                    for u in range(4):
                        for n in range(NT2):
                            nc.tensor.matmul(
                                po[n][:],
                                lhsT=hsegT[:, u * T : (u + 1) * T],
                                rhs=w2t[:, u * HIDDEN + n * 512 : u * HIDDEN + (n + 1) * 512],
                                start=False,
                                stop=False,
                                skip_group_check=True,
                            )

                # close the accumulation groups (stop is sim bookkeeping)
                for n in range(NT2):
                    nc.tensor.matmul(
                        po[n][:],
                        lhsT=zt[:, :P],
                        rhs=zt[:, :512],
                        start=False,
                        stop=True,
                        skip_group_check=True,
                    )

                osb = opool.tile([T, HIDDEN], F32, tag="osb")
                for n in range(NT2):
                    nc.scalar.copy(osb[:, n * 512 : (n + 1) * 512], po[n][:])
                nc.sync.dma_start(out_d[:, :], osb[:])

    nc.compile()
    return nc


def _get_nc(reps: int = 1):
    key = ("nc", reps)
    if key not in _CACHE:
        _CACHE[key] = _build(reps)
    return _CACHE[key]


def _prep_token_block(x_block: np.ndarray) -> np.ndarray:
    """[T, HIDDEN] f32 -> xt layout [P, KT1*T] bf16 where
    xt[p, k*T + t] = x_block[t, k*P + p]."""
    a = np.ascontiguousarray(
        x_block.T.reshape(KT1, P, T).transpose(1, 0, 2).reshape(P, KT1 * T)
    )
    return a.astype(ml_dtypes.bfloat16)


def _prep_w1(w1_e: np.ndarray) -> np.ndarray:
    """[HIDDEN, 2*INTER] f32 -> [P, NT1*GCOL] bf16 where
    [p, n*GCOL + k*512 + c] = w1_e[k*P + p, n*512 + c]."""
    a = w1_e.reshape(KT1, P, NT1, 512).transpose(1, 2, 0, 3)  # [p, n, k, c]
    return np.ascontiguousarray(a.reshape(P, NT1 * GCOL)).astype(
        ml_dtypes.bfloat16
    )


def _prep_w2(w2_e: np.ndarray) -> np.ndarray:
    """[INTER, HIDDEN] f32 -> [P, JT*HIDDEN] bf16 where
    [p, j*HIDDEN + c] = w2_e[j*P + p, c]."""
    return np.ascontiguousarray(
        w2_e.reshape(JT, P, HIDDEN).transpose(1, 0, 2).reshape(P, JT * HIDDEN)
    ).astype(ml_dtypes.bfloat16)


def _run_device(in_maps):
    nc = _get_nc()
    res = run_bass_kernel_spmd(nc, in_maps, core_ids=list(range(N_CORES)))
    return [r["out"] for r in res.results]


def kernel(permuted_tokens, w1, w2, tokens_per_expert):
    permuted_tokens = np.asarray(permuted_tokens, dtype=np.float32)
    w1 = np.asarray(w1, dtype=np.float32)
    w2 = np.asarray(w2, dtype=np.float32)
    counts = np.asarray(tokens_per_expert).astype(np.int64)

    n = permuted_tokens.shape[0]
    bounds = np.minimum(np.cumsum(counts), n)
    starts = np.concatenate([[0], bounds[:-1]])
    eff_counts = np.maximum(bounds - starts, 0)

    w1_maps = [_prep_w1(w1[e]) for e in range(EXPERTS)]
    w2_maps = [_prep_w2(w2[e]) for e in range(EXPERTS)]

    out = np.zeros((n, HIDDEN), dtype=np.float32)
    rounds = int(max(1, -(-int(eff_counts.max()) // T)))
    for r in range(rounds):
        in_maps = []
        chunk_info = []
        for e in range(EXPERTS):
            c0 = starts[e] + r * T
            cnt = int(min(max(eff_counts[e] - r * T, 0), T))
            blk = np.zeros((T, HIDDEN), dtype=np.float32)
            if cnt > 0:
                blk[:cnt] = permuted_tokens[c0 : c0 + cnt]
            chunk_info.append((c0, cnt))
            in_maps.append(
                {"xt": _prep_token_block(blk), "w1": w1_maps[e], "w2": w2_maps[e]}
            )
        outs = _run_device(in_maps)
        for e in range(EXPERTS):
            c0, cnt = chunk_info[e]
            if cnt > 0:
                out[c0 : c0 + cnt] = outs[e][:cnt]
    return out
